# revision 1
# baseline (speedup 1.0000x reference)
"""2-layer LSTM decoder for trn2 — 8-way tensor-parallel over the gate dim.

Everything lives transposed (feature dim on partitions, batch on the free
dim). Core c owns H-slice [128c:128(c+1)) of each gate: its stationary
weight tiles are (128 K-rows, 128 gate-cols) and the moving operand is
x/h.T (128, 512-batch) — full 512-wide matmuls, ~100 MM/step/core.
Hidden states are exchanged with one AllGather per layer per step; biases
ride the scalar-engine activation (per-partition bias vector).
"""
import numpy as np
import ml_dtypes


import concourse.bass as bass
import concourse.mybir as mybir
import concourse.tile as tile
from concourse import bacc

F32 = mybir.dt.float32
BF16 = mybir.dt.bfloat16
AF = mybir.ActivationFunctionType
ALU = mybir.AluOpType

B, T_FULL, F, H, GE = 512, 168, 32, 1024, 16
N_CORES = 8
NK0 = 9     # L0 contraction chunks (x-chunk + 8 h-chunks)
NKH = 8


def prep_host(inputs, T):
    inp = {k: np.asarray(v) for k, v in inputs.items()}
    gv_all = inp["group_emb"][inp["group_ids"].astype(np.int64)]  # (B, GE)
    b0 = (inp["b_ih0"] + inp["b_hh0"]).astype(np.float32)
    b1 = (inp["b_ih1"] + inp["b_hh1"]).astype(np.float32)

    w0_ext = np.zeros((NK0 * 128, 4096), np.float32)
    w0_ext[0:49] = inp["W_ih0"].astype(np.float32).T         # (49, 4096)
    w0_ext[128:1152] = inp["W_hh0"].astype(np.float32).T     # (1024, 4096)
    w1_ext = np.concatenate(
        [inp["W_ih1"].astype(np.float32).T, inp["W_hh1"].astype(np.float32).T],
        axis=0)                                              # (2048, 4096)

    # per-core stationary tiles: w[p, (k*4+m)*128 + j] = W_ext[128k+p, 1024m+128c+j]
    def slice_w(w_ext, nk, c):
        a = w_ext.reshape(nk, 128, 4, 8, 128)            # k p m c j
        return np.ascontiguousarray(
            a[:, :, :, c, :].transpose(1, 0, 2, 3).reshape(128, nk * 4 * 128)
        ).astype(ml_dtypes.bfloat16)

    b0r = b0.reshape(4, 8, 128)
    b1r = b1.reshape(4, 8, 128)
    wp = inp["W_proj"].astype(np.float32)[0]                 # (1024,)
    wpT = np.ascontiguousarray(wp.reshape(8, 128).T).astype(ml_dtypes.bfloat16)

    knT = np.ascontiguousarray(
        inp["dec_known"][:, :T, :].transpose(1, 2, 0)).astype(ml_dtypes.bfloat16)
    yT = np.ascontiguousarray(inp["target_y"][:, :T, 0].T).astype(ml_dtypes.bfloat16)
    gvT = np.ascontiguousarray(gv_all.T).astype(ml_dtypes.bfloat16)      # (16, 512)
    leT = np.ascontiguousarray(
        inp["last_enc_consumption"].T).astype(ml_dtypes.bfloat16)        # (1, 512)
    # hT_init: (2, 128, 8, 512): [l, p, k, b] = h0[l, b, 128k+p]
    hTi = np.ascontiguousarray(
        inp["h0"].astype(np.float32).reshape(2, B, 8, 128).transpose(0, 3, 2, 1)
    ).astype(ml_dtypes.bfloat16)

    shared = dict(knT=knT, yT=yT, gvT=gvT, leT=leT, h0Ti=hTi[0], h1Ti=hTi[1])
    per_core = []
    for c in range(N_CORES):
        d = dict(
            w0=slice_w(w0_ext, NK0, c),
            w1=slice_w(w1_ext, 2 * NKH, c),
            b0=np.ascontiguousarray(b0r[:, c, :].T).astype(np.float32),  # (128,4)
            b1=np.ascontiguousarray(b1r[:, c, :].T).astype(np.float32),
            wpT=wpT,
            c0i=np.ascontiguousarray(
                inp["c0"][0, :, 128 * c:128 * (c + 1)].T).astype(np.float32),
            c1i=np.ascontiguousarray(
                inp["c0"][1, :, 128 * c:128 * (c + 1)].T).astype(np.float32),
        )
        per_core.append(d)
    tf_mask = [int(v) for v in np.asarray(inp["tf_mask"]).reshape(-1)][:T]
    b_proj = float(np.asarray(inp["b_proj"]).reshape(-1)[0])
    return shared, per_core, tf_mask, b_proj


def build_module(T, tf_mask, b_proj, rep=1):
    nc = bacc.Bacc(target_bir_lowering=False)

    w0_d = nc.dram_tensor("w0", [128, NK0 * 512], BF16, kind="ExternalInput")
    w1_d = nc.dram_tensor("w1", [128, 2 * NKH * 512], BF16, kind="ExternalInput")
    b0_d = nc.dram_tensor("b0", [128, 4], F32, kind="ExternalInput")
    b1_d = nc.dram_tensor("b1", [128, 4], F32, kind="ExternalInput")
    wpT_d = nc.dram_tensor("wpT", [128, 8], BF16, kind="ExternalInput")
    knT_d = nc.dram_tensor("knT", [T, F, B], BF16, kind="ExternalInput")
    yT_d = nc.dram_tensor("yT", [T, B], BF16, kind="ExternalInput")
    gvT_d = nc.dram_tensor("gvT", [GE, B], BF16, kind="ExternalInput")
    leT_d = nc.dram_tensor("leT", [1, B], BF16, kind="ExternalInput")
    c0i_d = nc.dram_tensor("c0i", [128, B], F32, kind="ExternalInput")
    c1i_d = nc.dram_tensor("c1i", [128, B], F32, kind="ExternalInput")
    h0Ti_d = nc.dram_tensor("h0Ti", [128, NKH, B], BF16, kind="ExternalInput")
    h1Ti_d = nc.dram_tensor("h1Ti", [128, NKH, B], BF16, kind="ExternalInput")
    out_d = nc.dram_tensor("out", [T, B], F32, kind="ExternalOutput")

    RG = [list(range(N_CORES))]

    with tile.TileContext(nc) as tc:
        with tc.tile_pool(name="const", bufs=1) as const, \
             tc.tile_pool(name="hfp", bufs=2) as hfp, \
             tc.tile_pool(name="act", bufs=6) as actp, \
             tc.tile_pool(name="st", bufs=2) as stp, \
             tc.tile_pool(name="sm", bufs=2) as smp, \
             tc.tile_pool(name="gps", bufs=8, space="PSUM") as gpsum, \
             tc.tile_pool(name="dram", bufs=2, space="DRAM") as dramp:

            w0_sb = const.tile([128, NK0 * 512], BF16)
            nc.sync.dma_start(out=w0_sb[:], in_=w0_d[:])
            w1_sb = const.tile([128, 2 * NKH * 512], BF16)
            nc.sync.dma_start(out=w1_sb[:], in_=w1_d[:])
            b0_sb = const.tile([128, 4], F32)
            nc.sync.dma_start(out=b0_sb[:], in_=b0_d[:])
            b1_sb = const.tile([128, 4], F32)
            nc.sync.dma_start(out=b1_sb[:], in_=b1_d[:])
            wpT_sb = const.tile([128, 8], BF16)
            nc.sync.dma_start(out=wpT_sb[:], in_=wpT_d[:])

            xh0 = const.tile([128, B], BF16)
            nc.vector.memset(xh0[:], 0.0)
            nc.sync.dma_start(out=xh0[33:49, :], in_=gvT_d[:])

            def w0_slice(k, m):
                base = (k * 4 + m) * 128
                return w0_sb[:, base:base + 128]

            def w1_slice(k, m):
                base = (k * 4 + m) * 128
                return w1_sb[:, base:base + 128]

            for _rep in range(rep):
                c0_cur = stp.tile([128, B], F32, tag="c0", name=f"c0i_{_rep}")
                nc.sync.dma_start(out=c0_cur[:], in_=c0i_d[:])
                c1_cur = stp.tile([128, B], F32, tag="c1", name=f"c1i_{_rep}")
                nc.sync.dma_start(out=c1_cur[:], in_=c1i_d[:])
                h0f = hfp.tile([128, NKH, B], BF16, tag="h0f", name=f"h0i_{_rep}")
                nc.sync.dma_start(out=h0f[:], in_=h0Ti_d[:])
                h1f = hfp.tile([128, NKH, B], BF16, tag="h1f", name=f"h1i_{_rep}")
                nc.sync.dma_start(out=h1f[:], in_=h1Ti_d[:])
                pred_sb = None

                def cell(g, c_cur, b_sb, ctag, t):
                    """gate psum tiles g[0..3] (i,f,g,o) -> (h_slice bf16, c_new)."""
                    sig_i = actp.tile([128, B], BF16, tag="act", name=f"si_{ctag}_{t}")
                    nc.scalar.activation(sig_i[:], g[0][:], AF.Sigmoid,
                                         bias=b_sb[:, 0:1])
                    sig_f = actp.tile([128, B], BF16, tag="act", name=f"sf_{ctag}_{t}")
                    nc.scalar.activation(sig_f[:], g[1][:], AF.Sigmoid,
                                         bias=b_sb[:, 1:2])
                    tan_g = actp.tile([128, B], BF16, tag="act", name=f"tg_{ctag}_{t}")
                    nc.scalar.activation(tan_g[:], g[2][:], AF.Tanh,
                                         bias=b_sb[:, 2:3])
                    sig_o = actp.tile([128, B], BF16, tag="act", name=f"so_{ctag}_{t}")
                    nc.scalar.activation(sig_o[:], g[3][:], AF.Sigmoid,
                                         bias=b_sb[:, 3:4])
                    tmpf = stp.tile([128, B], F32, tag="tmpf", name=f"tf_{ctag}_{t}")
                    nc.vector.tensor_tensor(out=tmpf[:], in0=sig_f[:], in1=c_cur[:],
                                            op=ALU.mult)
                    tmpb = actp.tile([128, B], BF16, tag="act", name=f"tb_{ctag}_{t}")
                    nc.vector.tensor_tensor(out=tmpb[:], in0=sig_i[:], in1=tan_g[:],
                                            op=ALU.mult)
                    c_new = stp.tile([128, B], F32, tag=ctag, name=f"cn_{ctag}_{t}")
                    nc.vector.tensor_tensor(out=c_new[:], in0=tmpf[:], in1=tmpb[:],
                                            op=ALU.add)
                    tan_c = actp.tile([128, B], BF16, tag="act", name=f"tc_{ctag}_{t}")
                    nc.scalar.activation(tan_c[:], c_new[:], AF.Tanh)
                    hsl = stp.tile([128, B], BF16, tag=f"h_{ctag}",
                                   name=f"hs_{ctag}_{t}")
                    nc.vector.tensor_tensor(out=hsl[:], in0=sig_o[:], in1=tan_c[:],
                                            op=ALU.mult)
                    return hsl, c_new

                def allgather(hsl, tag, t):
                    cin = dramp.tile([128, B], BF16, tag=f"ci_{tag}",
                                     name=f"ci_{tag}_{t}")
                    cout = dramp.tile([NKH * 128, B], BF16, tag=f"co_{tag}",
                                      name=f"co_{tag}_{t}", addr_space="Shared")
                    nc.sync.dma_start(out=cin[:], in_=hsl[:])
                    nc.gpsimd.collective_compute(
                        "AllGather", ALU.bypass, ins=[cin[:]], outs=[cout[:]],
                        replica_groups=RG)
                    hf = hfp.tile([128, NKH, B], BF16, tag=tag, name=f"hf_{tag}_{t}")
                    for k in range(NKH):
                        nc.sync.dma_start(out=hf[:, k, :],
                                          in_=cout[128 * k:128 * (k + 1), :])
                    return hf

                def emit_pred(t):
                    # pred for step t from h1f (redundant on every core)
                    pp = gpsum.tile([1, B], F32, tag="g", name=f"pp_{t}")
                    for k in range(NKH):
                        nc.tensor.matmul(pp[:], wpT_sb[:, k:k + 1], h1f[:, k, :],
                                         start=(k == 0), stop=(k == NKH - 1))
                    ps = smp.tile([1, B], F32, tag="pred", name=f"pr_{t}")
                    nc.vector.tensor_scalar_add(ps[:], pp[:], b_proj)
                    nc.sync.dma_start(out=out_d[t:t + 1, :], in_=ps[:])
                    return ps

                for t in range(T):
                    kn_t = knT_d[t]
                    nc.sync.dma_start(out=xh0[1:33, :], in_=kn_t)

                    # --- L0 matmuls over h chunks first (pred of t-1 overlaps)
                    g0 = [gpsum.tile([128, B], F32, tag="g", name=f"g0_{t}_{m}")
                          for m in range(4)]
                    for k in range(1, NK0):
                        for m in range(4):
                            nc.tensor.matmul(g0[m][:], w0_slice(k, m),
                                             h0f[:, k - 1, :],
                                             start=(k == 1), stop=False)

                    # pred(t-1) + prev row, then the x chunk closes L0
                    if t == 0:
                        nc.sync.dma_start(out=xh0[0:1, :], in_=leT_d[:])
                    else:
                        pred_sb = emit_pred(t - 1)
                        if tf_mask[t - 1]:
                            nc.sync.dma_start(out=xh0[0:1, :], in_=yT_d[t - 1:t, :])
                        else:
                            nc.vector.tensor_copy(xh0[0:1, :], pred_sb[:])
                    for m in range(4):
                        nc.tensor.matmul(g0[m][:], w0_slice(0, m), xh0[:],
                                         start=False, stop=True)

                    h0sl, c0_cur = cell(g0, c0_cur, b0_sb, "c0", t)
                    h0f = allgather(h0sl, "h0f", t)

                    # --- L1 matmuls: hh chunks first (prev h1f), then ih chunks
                    g1 = [gpsum.tile([128, B], F32, tag="g", name=f"g1_{t}_{m}")
                          for m in range(4)]
                    for k in range(NKH):
                        for m in range(4):
                            nc.tensor.matmul(g1[m][:], w1_slice(NKH + k, m),
                                             h1f[:, k, :],
                                             start=(k == 0), stop=False)
                    for k in range(NKH):
                        for m in range(4):
                            nc.tensor.matmul(g1[m][:], w1_slice(k, m),
                                             h0f[:, k, :],
                                             start=False, stop=(k == NKH - 1))

                    h1sl, c1_cur = cell(g1, c1_cur, b1_sb, "c1", t)
                    h1f = allgather(h1sl, "h1f", t)
                emit_pred(T - 1)

    nc.finalize()
    return nc


def kernel(**inputs):
    from concourse.bass_utils import run_bass_kernel_spmd
    T = T_FULL
    shared, per_core, tf_mask, b_proj = prep_host(inputs, T)
    nc = build_module(T, tf_mask, b_proj)
    in_maps = []
    for c in range(N_CORES):
        m = dict(shared)
        m.update(per_core[c])
        in_maps.append(m)
    res = run_bass_kernel_spmd(nc, in_maps, list(range(N_CORES)))
    out = np.zeros((B, T, 1), np.float32)
    out[:, :, 0] = res.results[0]["out"].T
    return out

